# revision 16
# baseline (speedup 1.0000x reference)
"""Trainium2 Bass kernel for nn_DistributionLossWithLabel.

Reference computation (B=8192, C=64):
    lq = log(q); lp = log(p)
    positive[i] = mean_c p[i,c]*(lp[i,c]-lq[i,c])
    a[j]        = sum_c p[j,c]*lp[j,c] / C
    kl[i,j]     = a[j] - (lq @ p^T)[i,j] / C
    negative[i] = sum_j kl[i,j] + sum_j kl[i,j]*(1-L[i,j])
    loss        = sum_i positive[i]/negative[i]

Device reformulation (rows i sharded 8 ways, D = 2 - L):
    negative[i] = sum_j D[i,j]*kl[i,j]
                = (D@base)[i] - (1/C) sum_c dlq[i,c]*(D@p)[i,c]
    where lbar = mean_i lq[i,:], dlq = lq - lbar (small, kills the
    cancellation between the D@a and lq·Dp terms), and
    base[j] = a[j] - <lbar/C, p[j,:]> >= 0.

    [Dp | Dbase_hi | Dbase_lo] accumulates on the TensorEngine as
    paug^T @ D^T with BOTH operands fp8e4m3 in DoubleRow perf mode
    (2 contraction rows/cycle, 216ns per 512-col matmul), streaming
    D^T straight from HBM in contiguous >=2KB/partition slabs spread
    over the sync/scalar HWDGE rings + the gpsimd SWDGE ring (each
    ring sustains only ~100-170GB/s; together they reach the ~360GB/s
    HBM limit).  paug carries p scaled by 2^12 and base as an fp8
    hi/lo split so fp8 rounding noise averages out over the 8192-term
    row sums.  The 8192x8192 KL matrix never exists; the kernel is
    bound by reading D^T once (8MB/core).
"""

import sys

if "/opt/trn_rl_repo" not in sys.path:
    sys.path.insert(0, "/opt/trn_rl_repo")

import ml_dtypes
import numpy as np

import concourse.bass as bass
import concourse.tile as tile
from concourse import bacc, mybir
from concourse.masks import make_identity

FP = mybir.dt.float32
F16 = mybir.dt.float16
F8 = mybir.dt.float8e4
ALU = mybir.AluOpType
AX = mybir.AxisListType
PM = mybir.MatmulPerfMode

B_FULL = 8192
C = 64
N_CORES = 8
NAUG = 80  # 64 p cols + base_hi + base_lo + 14 pad (DoubleRow needs 16B-aligned chunk stride)
SP_P = 4096.0  # p scale into fp8 normal range
SP_BH = 4096.0  # base_hi scale
SP_BL = 131072.0  # base_lo scale (2^17)

# Label-tile DMA schedule: (queue, n_chunks) in matmul consumption order.
# Sized so all three queues finish together (sync ~170GB/s, scalar
# ~130GB/s, gpsimd SWDGE ~85GB/s + slow ramp); small leading tiles get
# the first matmuls started early.
SCHED = [
    ("sy", 4), ("sc", 4), ("gp", 4),
    ("sy", 4), ("sc", 4), ("gp", 4),
    ("sy", 4), ("sc", 4), ("gp", 4),
    ("sy", 4), ("sc", 4), ("gp", 4),
    ("sy", 4),
    ("sy", 2), ("sc", 2), ("gp", 2),
    ("sy", 2), ("sc", 2), ("gp", 2),
]
assert sum(n for _, n in SCHED) == 64


def build_nc(B=B_FULL, shard=B_FULL // N_CORES, debug=False):
    assert B % 512 == 0 and shard % 512 == 0
    njc = B // 128  # 128-row j-chunks
    nblk = shard // 128  # 128-row i-blocks
    nhalf = shard // 512
    ndch = njc // 2  # DoubleRow chunk pairs
    rcpC = 1.0 / C

    nc = bacc.Bacc("TRN2", target_bir_lowering=False, debug=debug)

    paug_d = nc.dram_tensor("paug", [128, njc * NAUG], F8, kind="ExternalInput")
    dlq_d = nc.dram_tensor("dlq", [128, nblk * 66], F16, kind="ExternalInput")
    pos_d = nc.dram_tensor("pos", [128, nblk], FP, kind="ExternalInput")
    lab_d = nc.dram_tensor("labels", [128, njc * shard], F8, kind="ExternalInput")
    out_d = nc.dram_tensor("out", [1, 1], FP, kind="ExternalOutput")

    with tile.TileContext(nc) as tc:
        with (
            tc.tile_pool(name="const", bufs=1) as cp,
            tc.tile_pool(name="spool", bufs=2) as sp,
            tc.tile_pool(name="mps_ps", bufs=1, space="PSUM") as mps_ps,
            tc.tile_pool(name="tr_ps", bufs=4, space="PSUM") as tr_ps,
            tc.tile_pool(name="acc_ps", bufs=1, space="PSUM") as acc_ps,
        ):
            paug = cp.tile([128, njc * NAUG], F8)
            LT = cp.tile([128, njc * shard], F8)
            dlq = cp.tile([128, nblk * 66], F16)
            pos = cp.tile([128, nblk], FP)
            ident = cp.tile([128, 128], FP)
            ones = cp.tile([128, 1], FP)

            # ---------------- DMA schedule ----------------
            lab_ap = lab_d.ap()
            paug_ap = paug_d.ap()
            engs = {"sy": nc.sync, "sc": nc.scalar, "gp": nc.gpsimd}
            nc.sync.dma_start(out=paug[:], in_=paug_ap)
            nc.gpsimd.dma_start(out=pos[:], in_=pos_d.ap())
            nc.gpsimd.dma_start(out=dlq[:], in_=dlq_d.ap())
            c0 = 0
            for qname, nch in SCHED:
                cs = slice(c0 * shard, (c0 + nch) * shard)
                engs[qname].dma_start(out=LT[:, cs], in_=lab_ap[:, cs])
                c0 += nch

            # Epilogue constants build after the gpsimd DMA issues so they
            # don't delay the SWDGE stream (only needed at epilogue time).
            make_identity(nc, ident[:])
            nc.gpsimd.memset(ones[:], 1.0)

            # ---------------- main loop: [Dp|Db]^T += paug^T @ D^T ----------
            LTv = LT[:].rearrange("p (n i) -> p n i", i=shard)
            paugv = paug[:].rearrange("p (n f) -> p n f", f=NAUG)
            mps = mps_ps.tile([128, shard], FP)
            for dch in range(ndch):
                w = paugv[:, 2 * dch : 2 * dch + 2, :]
                for h in range(nhalf):
                    i0 = h * 512
                    nc.tensor.matmul(
                        mps[0:NAUG, i0 : i0 + 512],
                        w,
                        LTv[:, 2 * dch : 2 * dch + 2, i0 : i0 + 512],
                        start=(dch == 0),
                        stop=(dch == ndch - 1),
                        perf_mode=PM.DoubleRow,
                    )

            # ---------------- epilogue (pipelined per 128-row block) -------
            DpT = cp.tile([128, shard], FP)
            updp = cp.tile([128, nblk], FP)
            half = shard // 2
            # two wide PSUM->SBUF copies on different engines in parallel
            # (one instruction has ~250ns fixed cost; per-block copies
            # serialize at ~400ns each)
            nc.scalar.copy(DpT[0:NAUG, 0:half], mps[0:NAUG, 0:half])
            nc.vector.tensor_copy(DpT[0:NAUG, half:], mps[0:NAUG, half:])
            for blk in range(nblk):
                bs = slice(blk * 128, (blk + 1) * 128)
                tr = tr_ps.tile([128, NAUG], FP, tag="tr")
                nc.tensor.transpose(
                    tr[:], DpT[0:NAUG, bs], ident[0:NAUG, 0:NAUG]
                )
                escr = sp.tile([128, 66], FP, tag="escr")
                # cols 0:64 of dlq are (lq-lbar); cols 64,65 are the
                # constants -64, -2 that fold the base hi/lo columns in:
                # updp = SP_BH*(T2 - Dbase) = -SP_BH*negative
                nc.vector.scalar_tensor_tensor(
                    out=escr[:],
                    in0=tr[:, 0:66],
                    scalar=rcpC * (SP_BH / SP_P),
                    in1=dlq[:, blk * 66 : (blk + 1) * 66],
                    op0=ALU.mult,
                    op1=ALU.mult,
                    accum_out=updp[:, blk : blk + 1],
                )
            rec8 = cp.tile([128, nblk], FP)
            nc.vector.reciprocal(rec8[:], updp[:])
            # pos is shipped as -positive*SP_BH, cancelling the sign of
            # updp = -SP_BH*negative: out_col = sum_blk pos/neg
            r8 = cp.tile([128, nblk], FP)
            out_col = cp.tile([128, 1], FP)
            nc.vector.scalar_tensor_tensor(
                out=r8[:],
                in0=pos[:],
                scalar=1.0,
                in1=rec8[:],
                op0=ALU.mult,
                op1=ALU.mult,
                accum_out=out_col[:],
            )
            # collapse 128 partitions -> single scalar on the PE so the
            # output DMA is one contiguous 4-byte descriptor.
            acc = acc_ps.tile([1, 1], FP)
            nc.tensor.matmul(acc[:], ones[:], out_col[:], start=True, stop=True)
            out_sb = cp.tile([1, 1], FP)
            nc.vector.tensor_copy(out_sb[:], acc[:])
            nc.sync.dma_start(out=out_d.ap(), in_=out_sb[:])

    nc.compile()
    return nc


_NC_CACHE = {}


def _get_nc(B, shard):
    key = (B, shard)
    if key not in _NC_CACHE:
        _NC_CACHE[key] = build_nc(B, shard)
    return _NC_CACHE[key]


def chunk_rows(arr):
    """[N, W] -> [128, (N/128)*W]: partition pp, col n*W+c = row n*128+pp."""
    n, w = arr.shape[0] // 128, arr.shape[1]
    return np.ascontiguousarray(
        arr.reshape(n, 128, w).transpose(1, 0, 2).reshape(128, n * w)
    )


def _f8(x):
    return np.asarray(x, np.float32).astype(ml_dtypes.float8_e4m3)


def make_in_maps(q, p, labels_matrix, n_cores=N_CORES):
    B, nC = q.shape
    shard = B // n_cores
    njc = B // 128
    nblk = shard // 128

    q64 = q.astype(np.float64)
    p64 = p.astype(np.float64)
    lq = np.log(q64)
    lp = np.log(p64)
    pos = (p64 * (lp - lq)).mean(axis=1)  # [B]
    a = (p64 * lp).sum(axis=1) / nC  # [B]
    lbar = lq.mean(axis=0)  # [C]
    base = a - p64 @ (lbar / nC)  # [B]
    # 66-col dlq: the two trailing constants fold the base hi/lo PSUM
    # columns into the per-block dot (scale rcpC*SP_BH/SP_P = 1/64):
    # -64 * psum_hi/64 = -SP_BH*Dbase_hi ; -2 * psum_lo/64 = -SP_BH*Dbase_lo/32
    dlq16 = np.empty((B, 66), dtype=np.float16)
    dlq16[:, 0:nC] = (lq - lbar).astype(np.float16)
    dlq16[:, nC] = -(nC * SP_P / SP_BH)
    dlq16[:, nC + 1] = -(nC * SP_P / SP_BL)

    b_hi8 = _f8(base * SP_BH)
    b_hi = b_hi8.astype(np.float64) / SP_BH
    b_lo8 = _f8((base - b_hi) * SP_BL)

    paug_full = np.zeros((B, NAUG), dtype=ml_dtypes.float8_e4m3)
    paug_full[:, 0:nC] = _f8(p64 * SP_P)
    paug_full[:, nC] = b_hi8
    paug_full[:, nC + 1] = b_lo8
    paug_ch = chunk_rows(paug_full)

    # D^T fully chunked: [128, njc, B] with [pp, n, i] = (2-L)[i, n*128+pp]
    Dt8 = _f8(2.0 - labels_matrix).T  # [B(j), B(i)]
    Dt_ch = Dt8.reshape(njc, 128, B).transpose(1, 0, 2)  # [128, njc, B]

    maps = []
    for k in range(n_cores):
        s = slice(k * shard, (k + 1) * shard)
        maps.append(
            {
                "paug": paug_ch,
                "dlq": chunk_rows(dlq16[s]),
                "pos": np.ascontiguousarray(
                    (-pos[s] * SP_BH).astype(np.float32).reshape(nblk, 128).T
                ),
                "labels": np.ascontiguousarray(
                    Dt_ch[:, :, s].reshape(128, njc * shard)
                ),
            }
        )
    return maps


def kernel(q, p, labels_matrix):
    from concourse.bass_utils import run_bass_kernel_spmd

    q = np.asarray(q, dtype=np.float32)
    p = np.asarray(p, dtype=np.float32)
    labels_matrix = np.asarray(labels_matrix, dtype=np.float32)
    B = q.shape[0]
    shard = B // N_CORES
    nc = _get_nc(B, shard)
    in_maps = make_in_maps(q, p, labels_matrix, N_CORES)
    res = run_bass_kernel_spmd(nc, in_maps, core_ids=list(range(N_CORES)))
    total = 0.0
    for r in res.results:
        total += r["out"].astype(np.float64).sum()
    return np.float32(total)
